# revision 5
# baseline (speedup 1.0000x reference)
"""MinLSTM fused kernel for Trainium2 (8 NeuronCores, SPMD).

Math: the reference applies cumlogsumexp over the sequence but only the LAST
timestep feeds the output head, so the scan collapses to a single logsumexp
reduction over sequence:

    log_h_last = log_f[S-1] + log(0.5 + sum_s exp(diff_s + log_g(h_s)))
    out = exp(log_h_last) @ w_out.T + b_out

with diff = softplus(-f) - softplus(-i) and per-token term

    exp(diff + log_g(h)) = (1 + e^{-f}) * sigmoid(i) * g(h)
                         = 1/4 * (1+e^{-f}) * (1+tanh(i/2)) * max(1+2h, 1+tanh(h/2))

which needs only {exp, tanh, copy} — all in the ACT `exp_and_others` table
(single table load). The device fuses the z = x @ w_in.T matmul (fp8
DoubleRow, fp32 PSUM accumulation) with the per-token nonlinearity and the
per-(batch, channel) partial sums. The host applies the exact last-token
correction in fp64 and runs the tiny [4,1024]x[1024,1024] output head.

Sharding: hidden-channel parallel — core c owns h-channels [c*128, (c+1)*128)
(i.e. 3 x 128 rows of w_in) and streams ALL 32768 tokens. This keeps the
per-core weight load to a single 384 KB DMA (vs streaming 3 MB of stripes),
makes every token block a full 512 (the moving-dim size where the PE is
stream-bound rather than LDWEIGHTS-bound), and leaves a single short
ACT->DVE drain after the last matmul. Per-core HBM traffic is 32 MB of fp8
x at ~195 GB/s sustained — well under the ~358 GB/s per-core ceiling.

Startup: weights and the first x block are DMA'd in kb-pair pieces on the
two HWDGE queues (w on scalar, x on sync) so the first real matmul can start
as soon as the first ~224 KB land, chasing the remaining pieces. A short
dummy-matmul burst opens the PE HAM clock gate early; the first real matmuls
still run partially cold (1.2 GHz) — cheaper than idling until warm.
"""

from contextlib import ExitStack

import ml_dtypes
import numpy as np

B, S, D, H = 4, 8192, 1024, 1024
N_CORES = 8
CH = H // N_CORES       # 128 h-channels per core
TOKS = B * S            # 32768 tokens, all streamed by every core
TB = 512                # token block (PSUM bank / moving free dim)
NTB = TOKS // TB        # 64
NBB = S // TB           # 16 blocks per batch sequence
KC = D // 128           # 8 contraction chunks of 128

USE_FP8 = True
WSCALE = 64.0           # w pre-scale so fp8 w values sit in the normal range
# HAM pre-warm matmuls issued while startup DMAs fly. The first ~224 KB of
# startup DMA can't land before ~3.5us after body entry (HWDGE issue + first
# byte + transfer + completion receipt), which matches the ~3.4us the PE HAM
# clock gate needs of sustained activity to reach 2.4 GHz — so burn the whole
# DMA wait on dummies (~53.5ns each cold) and start the real stream warm.
N_DUMMY = 60

_CACHE = {}


def _build_nc(use_fp8):
    import concourse.bacc as bacc
    import concourse.mybir as mybir
    import concourse.tile as tile

    dt = mybir.dt
    AF = mybir.ActivationFunctionType
    ALU = mybir.AluOpType

    in_dt = dt.float8e4 if use_fp8 else dt.bfloat16
    inv = 1.0 / WSCALE if use_fp8 else 1.0

    nc = bacc.Bacc("TRN2", target_bir_lowering=False)
    # xt[tb, p, kc, s] = x_flat[tb*TB + s, kc*128 + p] — per-partition rows
    # are KC*TB contiguous bytes so each block DMA is dense 4 KB descriptors.
    xt = nc.dram_tensor("xt", (NTB, 128, KC, TB), in_dt, kind="ExternalInput")
    # wt[p, kc, g*128+c] = w_in[g*H + core*128 + c, kc*128 + p]
    wt = nc.dram_tensor("wt", (128, KC, 384), in_dt, kind="ExternalInput")
    # sums[p, b] = sum over batch b's tokens of the per-channel term
    out_sums = nc.dram_tensor("sums", (128, B), dt.float32, kind="ExternalOutput")

    with tile.TileContext(nc) as tc, ExitStack() as ctx:
        wpool = ctx.enter_context(tc.tile_pool(name="w", bufs=1))
        xpool = ctx.enter_context(tc.tile_pool(name="x", bufs=3))
        gpool = ctx.enter_context(tc.tile_pool(name="g", bufs=4))
        spool = ctx.enter_context(tc.tile_pool(name="s", bufs=1))
        psh = ctx.enter_context(tc.tile_pool(name="psh", bufs=3, space="PSUM"))
        psf = ctx.enter_context(tc.tile_pool(name="psf", bufs=2, space="PSUM"))
        psi = ctx.enter_context(tc.tile_pool(name="psi", bufs=3, space="PSUM"))

        slab = spool.tile([128, B, NBB], dt.float32)

        # HAM pre-warm: PE clock gate defaults to 1.2 GHz and opens after
        # ~3.4us of sustained activity. Start the activity window during the
        # startup DMAs; the first real matmuls continue it (running cold is
        # still faster than waiting for warm).
        dum = gpool.tile([128, 64], dt.bfloat16, tag="dum")
        nc.vector.memset(dum[:], 0.0)
        psd = psh.tile([128, TB], dt.float32, tag="ps", bufs=3)
        for _ in range(N_DUMMY):
            nc.tensor.matmul(psd[0:64, 0:64], dum[:], dum[:], start=True, stop=True)

        # Startup-critical DMAs in kb-pair pieces across both HWDGE queues:
        # the first matmul only waits for the first ~224 KB.
        w_all = wpool.tile([128, KC, 384], in_dt)
        for kb in range(KC // 2):
            nc.scalar.dma_start(
                w_all[:, 2 * kb : 2 * kb + 2, :], wt[:, 2 * kb : 2 * kb + 2, :]
            )
        x0 = xpool.tile([128, KC, TB], in_dt, tag="x")
        for kb in range(KC // 2):
            nc.sync.dma_start(
                x0[:, 2 * kb : 2 * kb + 2, :], xt[0, :, 2 * kb : 2 * kb + 2, :]
            )

        for tb in range(NTB):
            bb, ib = divmod(tb, NBB)
            if tb == 0:
                x_sb = x0
            else:
                x_sb = xpool.tile([128, KC, TB], in_dt, tag="x")
                nc.sync.dma_start(x_sb[:], xt[tb])

            ph = psh.tile([128, TB], dt.float32, tag="ps", bufs=3)
            pf = psf.tile([128, TB], dt.float32, tag="ps", bufs=2)
            pi = psi.tile([128, TB], dt.float32, tag="ps", bufs=3)

            def mm(out_ap, g, kb, start, stop):
                nc.tensor.matmul(
                    out_ap,
                    w_all[:, 2 * kb : 2 * kb + 2, g * 128 : (g + 1) * 128],
                    x_sb[:, 2 * kb : 2 * kb + 2, :],
                    start=start,
                    stop=stop,
                    perf_mode=mybir.MatmulPerfMode.DoubleRow,
                    skip_group_check=(tb == 0),
                )

            # h first (feeds the ACT tanh + DVE max), i last (the only gate
            # on the post-last-matmul critical path: ti -> t -> readacc).
            # Block 0 interleaves the three gates kb-major so each arriving
            # x/w DMA piece feeds 3 matmuls (~650ns) — no chase stalls while
            # the startup pieces land ~300ns apart.
            if tb == 0:
                for kb in range(KC // 2):
                    for ps_t, g in ((ph, 2), (pf, 0), (pi, 1)):
                        mm(ps_t[:], g, kb, kb == 0, kb == KC // 2 - 1)
            else:
                for ps_t, g in ((ph, 2), (pf, 0), (pi, 1)):
                    for kb in range(KC // 2):
                        mm(ps_t[:], g, kb, kb == 0, kb == KC // 2 - 1)

            th = gpool.tile([128, TB], dt.bfloat16, tag="th")
            a = gpool.tile([128, TB], dt.bfloat16, tag="a")
            ti = gpool.tile([128, TB], dt.bfloat16, tag="ti")
            nc.scalar.activation(th[:], ph[:], AF.Tanh, scale=0.5 * inv)
            nc.scalar.activation(a[:], pf[:], AF.Exp, scale=-inv)
            nc.scalar.activation(ti[:], pi[:], AF.Tanh, scale=0.5 * inv)

            # m1 = max(2h, tanh(h/2));  w2 = 1 + m1;  r = (1+e^{-f}) * w2
            # t = (1+tanh(i/2)) * r, row-summed into slab[:, batch, block]
            m1 = gpool.tile([128, TB], dt.bfloat16, tag="m1")
            nc.vector.scalar_tensor_tensor(
                m1[:], ph[:], 2.0 * inv, th[:], op0=ALU.mult, op1=ALU.max
            )
            w2 = gpool.tile([128, TB], dt.bfloat16, tag="w2")
            nc.vector.tensor_scalar_add(w2[:], m1[:], 1.0)
            r = gpool.tile([128, TB], dt.bfloat16, tag="r")
            nc.vector.scalar_tensor_tensor(
                r[:], a[:], 1.0, w2[:], op0=ALU.add, op1=ALU.mult
            )
            t = gpool.tile([128, TB], dt.bfloat16, tag="t")
            nc.vector.scalar_tensor_tensor(
                t[:],
                ti[:],
                1.0,
                r[:],
                op0=ALU.add,
                op1=ALU.mult,
                accum_out=slab[:, bb, ib : ib + 1],
            )

        red = spool.tile([128, B], dt.float32)
        nc.vector.tensor_reduce(red[:], slab[:], axis=mybir.AxisListType.X, op=ALU.add)
        nc.sync.dma_start(out_sums[:], red[:])

    nc.compile()
    return nc


def _get_nc():
    key = "fp8" if USE_FP8 else "bf16"
    if key not in _CACHE:
        _CACHE[key] = _build_nc(USE_FP8)
    return _CACHE[key]


def _softplus(v):
    return np.log1p(np.exp(-np.abs(v))) + np.maximum(v, 0.0)


def kernel(x, w_in, w_out, b_out, _return_results=False, _trace=False):
    from concourse.bass_utils import run_bass_kernel_spmd

    x = np.asarray(x)
    w_in = np.asarray(w_in)
    w_out = np.asarray(w_out)
    b_out = np.asarray(b_out)

    if USE_FP8:
        cast_dt = ml_dtypes.float8_e4m3  # TRN FP8_EXP4: max ±240, inf above

        def cast(a):
            return np.clip(a, -240.0, 240.0).astype(cast_dt)

        w_scaled = w_in * WSCALE
    else:
        cast_dt = ml_dtypes.bfloat16

        def cast(a):
            return a.astype(cast_dt)

        w_scaled = w_in

    # per-core weight pack: wt[p, kc, g*128+c] = w_scaled[g*H + core*128+c, kc*128+p]
    w5 = w_scaled.reshape(3, N_CORES, CH, KC, 128)
    wts = []
    for c in range(N_CORES):
        wc = np.ascontiguousarray(w5[:, c].transpose(3, 2, 0, 1))  # [128p, KC, 3, CH]
        wts.append(np.asarray(cast(wc)).reshape(128, KC, 384))

    # shared token pack: xt[tb, p, kc, s] = x_flat[tb*TB + s, kc*128 + p]
    xq = cast(x.reshape(TOKS, D))
    xt = np.ascontiguousarray(
        np.asarray(xq).reshape(NTB, TB, KC, 128).transpose(0, 3, 2, 1)
    )

    in_maps = [{"xt": xt, "wt": wts[c]} for c in range(N_CORES)]

    nc = _get_nc()
    # the first execution of a freshly compiled NEFF occasionally hits a
    # transient NRT exec error on this setup — retry once
    try:
        res = run_bass_kernel_spmd(
            nc, in_maps, core_ids=list(range(N_CORES)), trace=_trace
        )
    except Exception:
        import time as _time

        _time.sleep(2.0)
        res = run_bass_kernel_spmd(
            nc, in_maps, core_ids=list(range(N_CORES)), trace=False
        )

    # sums[p, b] per core -> channel h = core*128 + p
    Ssum = (
        np.concatenate([np.asarray(r["sums"]).T for r in res.results], axis=1).astype(
            np.float64
        )
        * 0.25
    )  # [B, H]

    # exact last-token factor in fp64 (host): log_f[S-1] = -softplus(diff[S-1])
    z_last = x[:, -1, :].astype(np.float64) @ w_in.astype(np.float64).T
    f_l, i_l = z_last[:, :H], z_last[:, H : 2 * H]
    diff_l = _softplus(-f_l) - _softplus(-i_l)
    h_last = np.exp(-_softplus(diff_l) + np.log(0.5 + Ssum))
    out = (h_last @ w_out.astype(np.float64).T + b_out.astype(np.float64)).astype(
        np.float32
    )
    if _return_results:
        return out, res
    return out


# revision 7
# speedup vs baseline: 1.1110x; 1.1110x over previous
"""MinLSTM fused kernel for Trainium2 (8 NeuronCores, SPMD).

Math: the reference applies cumlogsumexp over the sequence but only the LAST
timestep feeds the output head, so the scan collapses to a single logsumexp
reduction over sequence:

    log_h_last = log_f[S-1] + log(0.5 + sum_s exp(diff_s + log_g(h_s)))
    out = exp(log_h_last) @ w_out.T + b_out

with diff = softplus(-f) - softplus(-i) and per-token term

    exp(diff + log_g(h)) = (1 + e^{-f}) * sigmoid(i) * g(h)
                         = 1/4 * (1+e^{-f}) * (1+tanh(i/2)) * max(1+2h, 1+tanh(h/2))

which needs only {exp, tanh, copy} — all in the ACT `exp_and_others` table
(single table load). The device fuses the z = x @ w_in.T matmul (fp8
DoubleRow, fp32 PSUM accumulation) with the per-token nonlinearity and the
per-(batch, channel) partial sums. The host applies the exact last-token
correction in fp64 and runs the tiny [4,1024]x[1024,1024] output head.

Sharding: hidden-channel parallel — core c owns h-channels [c*128, (c+1)*128)
(i.e. 3 x 128 rows of w_in) and streams ALL 32768 tokens. This keeps the
per-core weight load to a single 384 KB DMA (vs streaming 3 MB of stripes),
makes every token block a full 512 (the moving-dim size where the PE is
stream-bound rather than LDWEIGHTS-bound), and leaves a single short
ACT->DVE drain after the last matmul. Per-core HBM traffic is 32 MB of fp8
x at ~195 GB/s sustained — well under the ~358 GB/s per-core ceiling.

Startup: weights and the first x block are DMA'd in kb-pair pieces on the
two HWDGE queues (w on scalar, x on sync) so the first real matmul can start
as soon as the first ~224 KB land, chasing the remaining pieces. A short
dummy-matmul burst opens the PE HAM clock gate early; the first real matmuls
still run partially cold (1.2 GHz) — cheaper than idling until warm.
"""

from contextlib import ExitStack

import ml_dtypes
import numpy as np

B, S, D, H = 4, 8192, 1024, 1024
N_CORES = 8
CH = H // N_CORES       # 128 h-channels per core
TOKS = B * S            # 32768 tokens, all streamed by every core
TB = 512                # token block (PSUM bank / moving free dim)
NTB = TOKS // TB        # 64
NBB = S // TB           # 16 blocks per batch sequence
KC = D // 128           # 8 contraction chunks of 128

USE_FP8 = True
WSCALE = 64.0           # w pre-scale so fp8 w values sit in the normal range
# HAM pre-warm matmuls issued while startup DMAs fly. The first ~224 KB of
# startup DMA can't land before ~3.5us after body entry (HWDGE issue + first
# byte + transfer + completion receipt), which matches the ~3.4us the PE HAM
# clock gate needs of sustained activity to reach 2.4 GHz — so burn the whole
# DMA wait on dummies (~53.5ns each cold) and start the real stream warm.
N_DUMMY = 60

_CACHE = {}


def _build_nc(use_fp8):
    import concourse.bacc as bacc
    import concourse.mybir as mybir
    import concourse.tile as tile

    dt = mybir.dt
    AF = mybir.ActivationFunctionType
    ALU = mybir.AluOpType

    in_dt = dt.float8e4 if use_fp8 else dt.bfloat16
    inv = 1.0 / WSCALE if use_fp8 else 1.0

    nc = bacc.Bacc("TRN2", target_bir_lowering=False)
    # xt[tb, p, kc, s] = x_flat[tb*TB + s, kc*128 + p] — per-partition rows
    # are KC*TB contiguous bytes so each block DMA is dense 4 KB descriptors.
    xt = nc.dram_tensor("xt", (NTB, 128, KC, TB), in_dt, kind="ExternalInput")
    # wt[p, kc, g*128+c] = w_in[g*H + core*128 + c, kc*128 + p]
    wt = nc.dram_tensor("wt", (128, KC, 384), in_dt, kind="ExternalInput")
    # sums[p, b] = sum over batch b's tokens of the per-channel term
    out_sums = nc.dram_tensor("sums", (128, B), dt.float32, kind="ExternalOutput")

    with tile.TileContext(nc) as tc, ExitStack() as ctx:
        wpool = ctx.enter_context(tc.tile_pool(name="w", bufs=1))
        xpool = ctx.enter_context(tc.tile_pool(name="x", bufs=3))
        gpool = ctx.enter_context(tc.tile_pool(name="g", bufs=2))
        spool = ctx.enter_context(tc.tile_pool(name="s", bufs=1))
        psh = ctx.enter_context(tc.tile_pool(name="psh", bufs=2, space="PSUM"))
        psf = ctx.enter_context(tc.tile_pool(name="psf", bufs=2, space="PSUM"))
        psi = ctx.enter_context(tc.tile_pool(name="psi", bufs=2, space="PSUM"))

        slab = spool.tile([128, B, NBB], dt.float32)

        # HAM pre-warm: PE clock gate defaults to 1.2 GHz and opens after
        # ~3.4us of sustained activity. Start the activity window during the
        # startup DMAs; the first real matmuls continue it (running cold is
        # still faster than waiting for warm).
        dum = gpool.tile([128, 64], dt.bfloat16, tag="dum")
        nc.vector.memset(dum[:], 0.0)
        psd = psh.tile([128, TB], dt.float32, tag="ps", bufs=2)
        for _ in range(N_DUMMY):
            nc.tensor.matmul(psd[0:64, 0:64], dum[:], dum[:], start=True, stop=True)

        # Startup-critical DMAs in kb-pair pieces across both HWDGE queues:
        # the first matmul only waits for the first ~224 KB.
        w_all = wpool.tile([128, KC, 384], in_dt)
        for kb in range(KC // 2):
            nc.scalar.dma_start(
                w_all[:, 2 * kb : 2 * kb + 2, :], wt[:, 2 * kb : 2 * kb + 2, :]
            )
        x0 = xpool.tile([128, KC, TB], in_dt, tag="x")
        for kb in range(KC // 2):
            nc.sync.dma_start(
                x0[:, 2 * kb : 2 * kb + 2, :], xt[0, :, 2 * kb : 2 * kb + 2, :]
            )

        for tb in range(NTB):
            bb, ib = divmod(tb, NBB)
            if tb == 0:
                x_sb = x0
            else:
                x_sb = xpool.tile([128, KC, TB], in_dt, tag="x")
                nc.sync.dma_start(x_sb[:], xt[tb])

            ph = psh.tile([128, TB], dt.float32, tag="ps", bufs=2)
            pf = psf.tile([128, TB], dt.float32, tag="ps", bufs=2)
            pi = psi.tile([128, TB], dt.float32, tag="ps", bufs=2)

            def mm(out_ap, g, kb, start, stop):
                nc.tensor.matmul(
                    out_ap,
                    w_all[:, 2 * kb : 2 * kb + 2, g * 128 : (g + 1) * 128],
                    x_sb[:, 2 * kb : 2 * kb + 2, :],
                    start=start,
                    stop=stop,
                    perf_mode=mybir.MatmulPerfMode.DoubleRow,
                    skip_group_check=(tb == 0),
                )

            # h first (feeds the ACT tanh + DVE max), i last (the only gate
            # on the post-last-matmul critical path: ti -> t -> readacc).
            # Block 0 interleaves the three gates kb-major so each arriving
            # x/w DMA piece feeds 3 matmuls (~650ns) — no chase stalls while
            # the startup pieces land ~300ns apart.
            if tb == 0:
                for kb in range(KC // 2):
                    for ps_t, g in ((ph, 2), (pf, 0), (pi, 1)):
                        mm(ps_t[:], g, kb, kb == 0, kb == KC // 2 - 1)
            else:
                for ps_t, g in ((ph, 2), (pf, 0), (pi, 1)):
                    for kb in range(KC // 2):
                        mm(ps_t[:], g, kb, kb == 0, kb == KC // 2 - 1)

            th = gpool.tile([128, TB], dt.bfloat16, tag="th")
            h2 = gpool.tile([128, TB], dt.bfloat16, tag="h2")
            a = gpool.tile([128, TB], dt.bfloat16, tag="a")
            ti = gpool.tile([128, TB], dt.bfloat16, tag="ti")
            nc.scalar.activation(th[:], ph[:], AF.Tanh, scale=0.5 * inv)
            nc.scalar.activation(h2[:], ph[:], AF.Copy, scale=2.0 * inv, bias=1.0)
            nc.scalar.activation(a[:], pf[:], AF.Exp, scale=-inv)
            nc.scalar.activation(ti[:], pi[:], AF.Tanh, scale=0.5 * inv)

            # w2 = max(1+tanh(h/2), 1+2h);  r = (1+e^{-f}) * w2
            # t = (1+tanh(i/2)) * r, row-summed into slab[:, batch, block]
            w2 = gpool.tile([128, TB], dt.bfloat16, tag="w2")
            nc.vector.scalar_tensor_tensor(
                w2[:], th[:], 1.0, h2[:], op0=ALU.add, op1=ALU.max
            )
            r = gpool.tile([128, TB], dt.bfloat16, tag="r")
            nc.vector.scalar_tensor_tensor(
                r[:], a[:], 1.0, w2[:], op0=ALU.add, op1=ALU.mult
            )
            t = gpool.tile([128, TB], dt.bfloat16, tag="t")
            nc.vector.scalar_tensor_tensor(
                t[:],
                ti[:],
                1.0,
                r[:],
                op0=ALU.add,
                op1=ALU.mult,
                accum_out=slab[:, bb, ib : ib + 1],
            )

        red = spool.tile([128, B], dt.float32)
        nc.vector.tensor_reduce(red[:], slab[:], axis=mybir.AxisListType.X, op=ALU.add)
        nc.sync.dma_start(out_sums[:], red[:])

    nc.compile()
    return nc


def _get_nc():
    key = "fp8" if USE_FP8 else "bf16"
    if key not in _CACHE:
        _CACHE[key] = _build_nc(USE_FP8)
    return _CACHE[key]


def _softplus(v):
    return np.log1p(np.exp(-np.abs(v))) + np.maximum(v, 0.0)


def kernel(x, w_in, w_out, b_out, _return_results=False, _trace=False):
    from concourse.bass_utils import run_bass_kernel_spmd

    x = np.asarray(x)
    w_in = np.asarray(w_in)
    w_out = np.asarray(w_out)
    b_out = np.asarray(b_out)

    if USE_FP8:
        cast_dt = ml_dtypes.float8_e4m3  # TRN FP8_EXP4: max ±240, inf above

        def cast(a):
            return np.clip(a, -240.0, 240.0).astype(cast_dt)

        w_scaled = w_in * WSCALE
    else:
        cast_dt = ml_dtypes.bfloat16

        def cast(a):
            return a.astype(cast_dt)

        w_scaled = w_in

    # per-core weight pack: wt[p, kc, g*128+c] = w_scaled[g*H + core*128+c, kc*128+p]
    w5 = w_scaled.reshape(3, N_CORES, CH, KC, 128)
    wts = []
    for c in range(N_CORES):
        wc = np.ascontiguousarray(w5[:, c].transpose(3, 2, 0, 1))  # [128p, KC, 3, CH]
        wts.append(np.asarray(cast(wc)).reshape(128, KC, 384))

    # shared token pack: xt[tb, p, kc, s] = x_flat[tb*TB + s, kc*128 + p]
    xq = cast(x.reshape(TOKS, D))
    xt = np.ascontiguousarray(
        np.asarray(xq).reshape(NTB, TB, KC, 128).transpose(0, 3, 2, 1)
    )

    in_maps = [{"xt": xt, "wt": wts[c]} for c in range(N_CORES)]

    nc = _get_nc()
    # the first execution of a freshly compiled NEFF occasionally hits a
    # transient NRT exec error on this setup — retry once
    try:
        res = run_bass_kernel_spmd(
            nc, in_maps, core_ids=list(range(N_CORES)), trace=_trace
        )
    except Exception:
        import time as _time

        _time.sleep(2.0)
        res = run_bass_kernel_spmd(
            nc, in_maps, core_ids=list(range(N_CORES)), trace=False
        )

    # sums[p, b] per core -> channel h = core*128 + p
    Ssum = (
        np.concatenate([np.asarray(r["sums"]).T for r in res.results], axis=1).astype(
            np.float64
        )
        * 0.25
    )  # [B, H]

    # exact last-token factor in fp64 (host): log_f[S-1] = -softplus(diff[S-1])
    z_last = x[:, -1, :].astype(np.float64) @ w_in.astype(np.float64).T
    f_l, i_l = z_last[:, :H], z_last[:, H : 2 * H]
    diff_l = _softplus(-f_l) - _softplus(-i_l)
    h_last = np.exp(-_softplus(diff_l) + np.log(0.5 + Ssum))
    out = (h_last @ w_out.astype(np.float64).T + b_out.astype(np.float64)).astype(
        np.float32
    )
    if _return_results:
        return out, res
    return out


# revision 9
# speedup vs baseline: 1.1728x; 1.0557x over previous
"""MinLSTM fused kernel for Trainium2 (8 NeuronCores, SPMD).

Math: the reference applies cumlogsumexp over the sequence but only the LAST
timestep feeds the output head, so the scan collapses to a single logsumexp
reduction over sequence:

    log_h_last = log_f[S-1] + log(0.5 + sum_s exp(diff_s + log_g(h_s)))
    out = exp(log_h_last) @ w_out.T + b_out

with diff = softplus(-f) - softplus(-i) and per-token term

    exp(diff + log_g(h)) = (1 + e^{-f}) * sigmoid(i) * g(h)
                         = 1/4 * (1+e^{-f}) * (1+tanh(i/2)) * max(1+2h, 1+tanh(h/2))

which needs only {exp, tanh, copy} — all in the ACT `exp_and_others` table
(single table load). The device fuses the z = x @ w_in.T matmul (fp8
DoubleRow, fp32 PSUM accumulation) with the per-token nonlinearity and the
per-(batch, channel) partial sums. The host applies the exact last-token
correction in fp64 and runs the tiny [4,1024]x[1024,1024] output head.

Sharding: hidden-channel parallel — core c owns h-channels [c*128, (c+1)*128)
(i.e. 3 x 128 rows of w_in) and streams ALL 32768 tokens. This keeps the
per-core weight load to a single 384 KB DMA (vs streaming 3 MB of stripes),
makes every token block a full 512 (the moving-dim size where the PE is
stream-bound rather than LDWEIGHTS-bound), and leaves a single short
ACT->DVE drain after the last matmul. Per-core HBM traffic is 32 MB of fp8
x at ~195 GB/s sustained — well under the ~358 GB/s per-core ceiling.

Engine budget per 512-token block (PE period 2.59us):
  ACT: one 1024-elem tanh over the (i,h) 2-bank PSUM pair + copy(2h+1) +
       exp(-f)  ~2.1us
  DVE: 3 bf16 scalar_tensor_tensor + accumulator read  ~2.2us
PSUM reads stay on ACT only — a DVE op reading PSUM measurably saturates
the DVE and causes periodic PE write-after-read stalls.

Startup: weights and the first x block are DMA'd in kb-pair pieces on the
two HWDGE queues (w on scalar, x on sync); a dummy-matmul burst keeps the
PE busy so the HAM clock gate is open (2.4 GHz) right when the first pieces
land (~3.4us after body entry — DMA completion receipt and the HAM warmup
window happen to coincide).
"""

from contextlib import ExitStack

import ml_dtypes
import numpy as np

B, S, D, H = 4, 8192, 1024, 1024
N_CORES = 8
CH = H // N_CORES       # 128 h-channels per core
TOKS = B * S            # 32768 tokens, all streamed by every core
TB = 512                # token block (PSUM bank / moving free dim)
NTB = TOKS // TB        # 64
NBB = S // TB           # 16 blocks per batch sequence
KC = D // 128           # 8 contraction chunks of 128

USE_FP8 = True
WSCALE = 64.0           # w pre-scale so fp8 w values sit in the normal range
N_DUMMY = 60            # HAM pre-warm matmuls issued while startup DMAs fly

_CACHE = {}


def _build_nc(use_fp8):
    import concourse.bacc as bacc
    import concourse.mybir as mybir
    import concourse.tile as tile

    dt = mybir.dt
    AF = mybir.ActivationFunctionType
    ALU = mybir.AluOpType

    in_dt = dt.float8e4 if use_fp8 else dt.bfloat16
    inv = 1.0 / WSCALE if use_fp8 else 1.0

    nc = bacc.Bacc("TRN2", target_bir_lowering=False)
    # xt[tb, p, kc, s] = x_flat[tb*TB + s, kc*128 + p] — per-partition rows
    # are KC*TB contiguous bytes so each block DMA is dense 4 KB descriptors.
    xt = nc.dram_tensor("xt", (NTB, 128, KC, TB), in_dt, kind="ExternalInput")
    # wt[p, kc, g*128+c] = w_in[g*H + core*128 + c, kc*128 + p]
    wt = nc.dram_tensor("wt", (128, KC, 384), in_dt, kind="ExternalInput")
    # sums[p, b, 0] = partial over seq blocks 0..14, [p, b, 1] = block 15
    # (split so only the last block's cell rides the post-stream tail)
    out_sums = nc.dram_tensor("sums", (128, B, 2), dt.float32, kind="ExternalOutput")

    with tile.TileContext(nc) as tc, ExitStack() as ctx:
        wpool = ctx.enter_context(tc.tile_pool(name="w", bufs=1))
        # x prefetch depth: the block-k+N DMA can only ISSUE once block k's
        # matmuls retire (write-after-read) and then needs ~2.2us to land
        # (HWDGE issue + transfer + completion receipt). bufs=3 leaves zero
        # slack — one hiccup phase-locks the whole stream into periodic
        # PE-waits-for-x stalls. bufs=5 gives ~3 blocks (~7.8us) of slack.
        xpool = ctx.enter_context(tc.tile_pool(name="x", bufs=5))
        gpool = ctx.enter_context(tc.tile_pool(name="g", bufs=2))
        spool = ctx.enter_context(tc.tile_pool(name="s", bufs=1))
        psih = ctx.enter_context(tc.tile_pool(name="psih", bufs=2, space="PSUM"))
        psf = ctx.enter_context(tc.tile_pool(name="psf", bufs=2, space="PSUM"))

        slab = spool.tile([128, B, NBB], dt.float32)

        # HAM pre-warm: the PE clock gate defaults to 1.2 GHz and opens after
        # ~3.4us of sustained activity; the startup DMAs need about that long
        # to land. Keep the PE busy so the real stream starts warm.
        dum = gpool.tile([128, 64], dt.bfloat16, tag="dum")
        nc.vector.memset(dum[:], 0.0)
        psd = psih.tile([128, 2, TB], dt.float32, tag="ps", bufs=2)
        for _ in range(N_DUMMY):
            nc.tensor.matmul(psd[0:64, 0, 0:64], dum[:], dum[:], start=True, stop=True)

        # Startup-critical DMAs in kb-pair pieces across both HWDGE queues:
        # the first matmul only waits for the first ~224 KB.
        w_all = wpool.tile([128, KC, 384], in_dt)
        for kb in range(KC // 2):
            nc.scalar.dma_start(
                w_all[:, 2 * kb : 2 * kb + 2, :], wt[:, 2 * kb : 2 * kb + 2, :]
            )
        x0 = xpool.tile([128, KC, TB], in_dt, tag="x")
        for kb in range(KC // 2):
            nc.sync.dma_start(
                x0[:, 2 * kb : 2 * kb + 2, :], xt[0, :, 2 * kb : 2 * kb + 2, :]
            )

        red = spool.tile([128, B], dt.float32)

        for tb in range(NTB):
            bb, ib = divmod(tb, NBB)
            last = tb == NTB - 1
            if tb == 0:
                x_sb = x0
            else:
                x_sb = xpool.tile([128, KC, TB], in_dt, tag="x")
                nc.sync.dma_start(x_sb[:], xt[tb])

            pih = psih.tile([128, 2, TB], dt.float32, tag="ps", bufs=2)
            pf = psf.tile([128, TB], dt.float32, tag="ps", bufs=2)

            def mm(out_ap, g, kb, start, stop):
                nc.tensor.matmul(
                    out_ap,
                    w_all[:, 2 * kb : 2 * kb + 2, g * 128 : (g + 1) * 128],
                    x_sb[:, 2 * kb : 2 * kb + 2, :],
                    start=start,
                    stop=stop,
                    perf_mode=mybir.MatmulPerfMode.DoubleRow,
                    skip_group_check=(tb == 0),
                )

            # gate -> (psum target, weight column group): f=0, i=1, h=2
            tgt = {"h": (pih[:, 1, :], 2), "i": (pih[:, 0, :], 1), "f": (pf[:], 0)}
            if tb == 0:
                # interleave gates kb-major so each arriving startup DMA piece
                # feeds 3 matmuls — no stalls while pieces land ~300ns apart
                for kb in range(KC // 2):
                    for gate in ("h", "i", "f"):
                        ap, g = tgt[gate]
                        mm(ap, g, kb, kb == 0, kb == KC // 2 - 1)
            else:
                # h first (feeds copy + the i,h tanh pair), f last on the
                # final block handled below; mid-stream order h, i, f staggers
                # the three ACT ops at 1/3, 2/3, 3/3 of the block
                order = ("h", "f", "i") if last else ("h", "i", "f")
                for gate in order:
                    ap, g = tgt[gate]
                    for kb in range(KC // 2):
                        mm(ap, g, kb, kb == 0, kb == KC // 2 - 1)

            h2 = gpool.tile([128, TB], dt.bfloat16, tag="h2")
            a = gpool.tile([128, TB], dt.bfloat16, tag="a")
            nc.scalar.activation(h2[:], pih[:, 1, :], AF.Copy, scale=2.0 * inv, bias=1.0)
            if last:
                # split tanh so the post-last-matmul chain is just
                # tanh(i) -> t -> accumulator read -> tiny DMA
                th = gpool.tile([128, TB], dt.bfloat16, tag="th")
                ti = gpool.tile([128, TB], dt.bfloat16, tag="ti")
                nc.scalar.activation(th[:], pih[:, 1, :], AF.Tanh, scale=0.5 * inv)
                nc.scalar.activation(a[:], pf[:], AF.Exp, scale=-inv)
                nc.scalar.activation(ti[:], pih[:, 0, :], AF.Tanh, scale=0.5 * inv)
                t_i, t_h = ti, th
            else:
                tith = gpool.tile([128, 2, TB], dt.bfloat16, tag="tith")
                nc.scalar.activation(tith[:], pih[:], AF.Tanh, scale=0.5 * inv)
                nc.scalar.activation(a[:], pf[:], AF.Exp, scale=-inv)
                t_i, t_h = tith[:, 0], tith[:, 1]

            # w2 = max(1+tanh(h/2), 1+2h);  r = (1+e^{-f}) * w2
            # t = (1+tanh(i/2)) * r, row-summed into slab[:, batch, block]
            w2 = gpool.tile([128, TB], dt.bfloat16, tag="w2")
            nc.vector.scalar_tensor_tensor(
                w2[:], t_h[:], 1.0, h2[:], op0=ALU.add, op1=ALU.max
            )
            r = gpool.tile([128, TB], dt.bfloat16, tag="r")
            nc.vector.scalar_tensor_tensor(
                r[:], a[:], 1.0, w2[:], op0=ALU.add, op1=ALU.mult
            )
            t = gpool.tile([128, TB], dt.bfloat16, tag="t")
            nc.vector.scalar_tensor_tensor(
                t[:],
                t_i[:],
                1.0,
                r[:],
                op0=ALU.add,
                op1=ALU.mult,
                accum_out=slab[:, bb, ib : ib + 1],
            )

            if tb == NTB - 2:
                # everything except the final block's cell is ready now:
                # reduce + ship it while the last block still streams
                nc.vector.tensor_reduce(
                    red[:], slab[:, :, 0 : NBB - 1], axis=mybir.AxisListType.X,
                    op=ALU.add,
                )
                nc.sync.dma_start(out_sums[:, :, 0], red[:])

        nc.sync.dma_start(out_sums[:, :, 1], slab[:, :, NBB - 1])

    nc.compile()
    return nc


def _get_nc():
    key = "fp8" if USE_FP8 else "bf16"
    if key not in _CACHE:
        _CACHE[key] = _build_nc(USE_FP8)
    return _CACHE[key]


def _softplus(v):
    return np.log1p(np.exp(-np.abs(v))) + np.maximum(v, 0.0)


def kernel(x, w_in, w_out, b_out, _return_results=False, _trace=False):
    from concourse.bass_utils import run_bass_kernel_spmd

    x = np.asarray(x)
    w_in = np.asarray(w_in)
    w_out = np.asarray(w_out)
    b_out = np.asarray(b_out)

    if USE_FP8:
        cast_dt = ml_dtypes.float8_e4m3  # TRN FP8_EXP4: max ±240, inf above

        def cast(a):
            return np.clip(a, -240.0, 240.0).astype(cast_dt)

        w_scaled = w_in * WSCALE
    else:
        cast_dt = ml_dtypes.bfloat16

        def cast(a):
            return a.astype(cast_dt)

        w_scaled = w_in

    # per-core weight pack: wt[p, kc, g*128+c] = w_scaled[g*H + core*128+c, kc*128+p]
    w5 = w_scaled.reshape(3, N_CORES, CH, KC, 128)
    wts = []
    for c in range(N_CORES):
        wc = np.ascontiguousarray(w5[:, c].transpose(3, 2, 0, 1))  # [128p, KC, 3, CH]
        wts.append(np.asarray(cast(wc)).reshape(128, KC, 384))

    # shared token pack: xt[tb, p, kc, s] = x_flat[tb*TB + s, kc*128 + p]
    xq = cast(x.reshape(TOKS, D))
    xt = np.ascontiguousarray(
        np.asarray(xq).reshape(NTB, TB, KC, 128).transpose(0, 3, 2, 1)
    )

    in_maps = [{"xt": xt, "wt": wts[c]} for c in range(N_CORES)]

    nc = _get_nc()
    # the first execution of a freshly compiled NEFF occasionally hits a
    # transient NRT exec error on this setup — retry once
    try:
        res = run_bass_kernel_spmd(
            nc, in_maps, core_ids=list(range(N_CORES)), trace=_trace
        )
    except Exception:
        import time as _time

        _time.sleep(2.0)
        res = run_bass_kernel_spmd(
            nc, in_maps, core_ids=list(range(N_CORES)), trace=False
        )

    # sums[p, b, :].sum(-1) per core -> channel h = core*128 + p
    Ssum = (
        np.concatenate(
            [np.asarray(r["sums"]).astype(np.float64).sum(axis=2).T for r in res.results],
            axis=1,
        )
        * 0.25
    )  # [B, H]

    # exact last-token factor in fp64 (host): log_f[S-1] = -softplus(diff[S-1])
    z_last = x[:, -1, :].astype(np.float64) @ w_in.astype(np.float64).T
    f_l, i_l = z_last[:, :H], z_last[:, H : 2 * H]
    diff_l = _softplus(-f_l) - _softplus(-i_l)
    h_last = np.exp(-_softplus(diff_l) + np.log(0.5 + Ssum))
    out = (h_last @ w_out.astype(np.float64).T + b_out.astype(np.float64)).astype(
        np.float32
    )
    if _return_results:
        return out, res
    return out


# revision 10
# speedup vs baseline: 1.1861x; 1.0113x over previous
"""MinLSTM fused kernel for Trainium2 (8 NeuronCores, SPMD).

Math: the reference applies cumlogsumexp over the sequence but only the LAST
timestep feeds the output head, so the scan collapses to a single logsumexp
reduction over sequence:

    log_h_last = log_f[S-1] + log(0.5 + sum_s exp(diff_s + log_g(h_s)))
    out = exp(log_h_last) @ w_out.T + b_out

with diff = softplus(-f) - softplus(-i) and per-token term

    exp(diff + log_g(h)) = (1 + e^{-f}) * sigmoid(i) * g(h)
                         = 1/4 * (1+e^{-f}) * (1+tanh(i/2)) * max(1+2h, 1+tanh(h/2))

which needs only {exp, tanh, copy} — all in the ACT `exp_and_others` table
(single table load). The device fuses the z = x @ w_in.T matmul (fp8
DoubleRow, fp32 PSUM accumulation) with the per-token nonlinearity and the
per-(batch, channel) partial sums. The host applies the exact last-token
correction in fp64 and runs the tiny [4,1024]x[1024,1024] output head.

Sharding: hidden-channel parallel — core c owns h-channels [c*128, (c+1)*128)
(i.e. 3 x 128 rows of w_in) and streams ALL 32768 tokens. This keeps the
per-core weight load to a single 384 KB DMA (vs streaming 3 MB of stripes),
makes every token block a full 512 (the moving-dim size where the PE is
stream-bound rather than LDWEIGHTS-bound), and leaves a single short
ACT->DVE drain after the last matmul. Per-core HBM traffic is 32 MB of fp8
x at ~195 GB/s sustained — well under the ~358 GB/s per-core ceiling.

Engine budget per 512-token block (PE period 2.59us):
  ACT: one 1024-elem tanh over the (i,h) 2-bank PSUM pair + copy(2h+1) +
       exp(-f)  ~2.1us
  DVE: 3 bf16 scalar_tensor_tensor + accumulator read  ~2.2us
PSUM reads stay on ACT only — a DVE op reading PSUM measurably saturates
the DVE and causes periodic PE write-after-read stalls.

Startup: weights and the first x block are DMA'd in kb-pair pieces on the
two HWDGE queues (w on scalar, x on sync); a dummy-matmul burst keeps the
PE busy so the HAM clock gate is open (2.4 GHz) right when the first pieces
land (~3.4us after body entry — DMA completion receipt and the HAM warmup
window happen to coincide).
"""

from contextlib import ExitStack

import ml_dtypes
import numpy as np

B, S, D, H = 4, 8192, 1024, 1024
N_CORES = 8
CH = H // N_CORES       # 128 h-channels per core
TOKS = B * S            # 32768 tokens, all streamed by every core
TB = 512                # token block (PSUM bank / moving free dim)
NTB = TOKS // TB        # 64
NBB = S // TB           # 16 blocks per batch sequence
KC = D // 128           # 8 contraction chunks of 128

USE_FP8 = True
WSCALE = 64.0           # w pre-scale so fp8 w values sit in the normal range
N_DUMMY = 60            # HAM pre-warm matmuls issued while startup DMAs fly

_CACHE = {}


def _build_nc(use_fp8):
    import concourse.bacc as bacc
    import concourse.mybir as mybir
    import concourse.tile as tile

    dt = mybir.dt
    AF = mybir.ActivationFunctionType
    ALU = mybir.AluOpType

    in_dt = dt.float8e4 if use_fp8 else dt.bfloat16
    inv = 1.0 / WSCALE if use_fp8 else 1.0

    nc = bacc.Bacc("TRN2", target_bir_lowering=False)
    # xt[tb, p, kc, s] = x_flat[tb*TB + s, kc*128 + p] — per-partition rows
    # are KC*TB contiguous bytes so each block DMA is dense 4 KB descriptors.
    xt = nc.dram_tensor("xt", (NTB, 128, KC, TB), in_dt, kind="ExternalInput")
    # wt[p, kc, g*128+c] = w_in[g*H + core*128 + c, kc*128 + p]
    wt = nc.dram_tensor("wt", (128, KC, 384), in_dt, kind="ExternalInput")
    # split outputs so only the final seq-block's cells ride the post-stream
    # tail, and both DMAs are fully contiguous (a strided slice DMA degrades
    # to 4-byte descriptors with ~5us completion latency)
    out_a = nc.dram_tensor("sums_a", (128, B, NBB - 1), dt.float32, kind="ExternalOutput")
    out_b = nc.dram_tensor("sums_b", (128, B), dt.float32, kind="ExternalOutput")

    with tile.TileContext(nc) as tc, ExitStack() as ctx:
        wpool = ctx.enter_context(tc.tile_pool(name="w", bufs=1))
        # x prefetch depth: the block-k+N DMA can only ISSUE once block k's
        # matmuls retire (write-after-read) and then needs ~2.2us to land
        # (HWDGE issue + transfer + completion receipt). bufs=3 leaves zero
        # slack — one hiccup phase-locks the whole stream into periodic
        # PE-waits-for-x stalls. bufs=5 gives ~3 blocks (~7.8us) of slack.
        xpool = ctx.enter_context(tc.tile_pool(name="x", bufs=5))
        gpool = ctx.enter_context(tc.tile_pool(name="g", bufs=2))
        spool = ctx.enter_context(tc.tile_pool(name="s", bufs=1))
        psih = ctx.enter_context(tc.tile_pool(name="psih", bufs=2, space="PSUM"))
        psf = ctx.enter_context(tc.tile_pool(name="psf", bufs=2, space="PSUM"))

        slab_a = spool.tile([128, B, NBB - 1], dt.float32)
        slab_b = spool.tile([128, B], dt.float32)

        # HAM pre-warm: the PE clock gate defaults to 1.2 GHz and opens after
        # ~3.4us of sustained activity; the startup DMAs need about that long
        # to land. Keep the PE busy so the real stream starts warm.
        dum = gpool.tile([128, 64], dt.bfloat16, tag="dum")
        nc.vector.memset(dum[:], 0.0)
        psd = psih.tile([128, 2, TB], dt.float32, tag="ps", bufs=2)
        for _ in range(N_DUMMY):
            nc.tensor.matmul(psd[0:64, 0, 0:64], dum[:], dum[:], start=True, stop=True)

        # Startup-critical DMAs in kb-pair pieces across both HWDGE queues:
        # the first matmul only waits for the first ~224 KB.
        w_all = wpool.tile([128, KC, 384], in_dt)
        for kb in range(KC // 2):
            nc.scalar.dma_start(
                w_all[:, 2 * kb : 2 * kb + 2, :], wt[:, 2 * kb : 2 * kb + 2, :]
            )
        x0 = xpool.tile([128, KC, TB], in_dt, tag="x")
        for kb in range(KC // 2):
            nc.sync.dma_start(
                x0[:, 2 * kb : 2 * kb + 2, :], xt[0, :, 2 * kb : 2 * kb + 2, :]
            )

        for tb in range(NTB):
            bb, ib = divmod(tb, NBB)
            last = tb == NTB - 1
            if tb == 0:
                x_sb = x0
            else:
                x_sb = xpool.tile([128, KC, TB], in_dt, tag="x")
                nc.sync.dma_start(x_sb[:], xt[tb])

            pih = psih.tile([128, 2, TB], dt.float32, tag="ps", bufs=2)
            pf = psf.tile([128, TB], dt.float32, tag="ps", bufs=2)

            def mm(out_ap, g, kb, start, stop):
                nc.tensor.matmul(
                    out_ap,
                    w_all[:, 2 * kb : 2 * kb + 2, g * 128 : (g + 1) * 128],
                    x_sb[:, 2 * kb : 2 * kb + 2, :],
                    start=start,
                    stop=stop,
                    perf_mode=mybir.MatmulPerfMode.DoubleRow,
                    skip_group_check=(tb == 0),
                )

            # gate -> (psum target, weight column group): f=0, i=1, h=2
            tgt = {"h": (pih[:, 1, :], 2), "i": (pih[:, 0, :], 1), "f": (pf[:], 0)}
            if tb == 0:
                # interleave gates kb-major so each arriving startup DMA piece
                # feeds 3 matmuls — no stalls while pieces land ~300ns apart
                for kb in range(KC // 2):
                    for gate in ("h", "i", "f"):
                        ap, g = tgt[gate]
                        mm(ap, g, kb, kb == 0, kb == KC // 2 - 1)
            else:
                # h first (feeds copy + the i,h tanh pair), f last on the
                # final block handled below; mid-stream order h, i, f staggers
                # the three ACT ops at 1/3, 2/3, 3/3 of the block
                order = ("h", "f", "i") if last else ("h", "i", "f")
                for gate in order:
                    ap, g = tgt[gate]
                    for kb in range(KC // 2):
                        mm(ap, g, kb, kb == 0, kb == KC // 2 - 1)

            h2 = gpool.tile([128, TB], dt.bfloat16, tag="h2")
            a = gpool.tile([128, TB], dt.bfloat16, tag="a")
            if last:
                # keep the final block's ACT queue short: h2 moves to the DVE
                # (a one-off PSUM read is fine; only steady-state DVE-PSUM
                # reads saturate), tanh is split so the post-last-matmul
                # chain is just tanh(i) -> t -> accumulator read -> tiny DMA
                nc.vector.tensor_scalar(
                    h2[:], pih[:, 1, :], 2.0 * inv, 1.0, op0=ALU.mult, op1=ALU.add
                )
                th = gpool.tile([128, TB], dt.bfloat16, tag="th")
                ti = gpool.tile([128, TB], dt.bfloat16, tag="ti")
                nc.scalar.activation(th[:], pih[:, 1, :], AF.Tanh, scale=0.5 * inv)
                nc.scalar.activation(a[:], pf[:], AF.Exp, scale=-inv)
                nc.scalar.activation(ti[:], pih[:, 0, :], AF.Tanh, scale=0.5 * inv)
                t_i, t_h = ti, th
            else:
                nc.scalar.activation(
                    h2[:], pih[:, 1, :], AF.Copy, scale=2.0 * inv, bias=1.0
                )
                tith = gpool.tile([128, 2, TB], dt.bfloat16, tag="tith")
                nc.scalar.activation(tith[:], pih[:], AF.Tanh, scale=0.5 * inv)
                nc.scalar.activation(a[:], pf[:], AF.Exp, scale=-inv)
                t_i, t_h = tith[:, 0], tith[:, 1]

            # w2 = max(1+tanh(h/2), 1+2h);  r = (1+e^{-f}) * w2
            # t = (1+tanh(i/2)) * r, row-summed into slab[:, batch, block]
            w2 = gpool.tile([128, TB], dt.bfloat16, tag="w2")
            nc.vector.scalar_tensor_tensor(
                w2[:], t_h[:], 1.0, h2[:], op0=ALU.add, op1=ALU.max
            )
            r = gpool.tile([128, TB], dt.bfloat16, tag="r")
            nc.vector.scalar_tensor_tensor(
                r[:], a[:], 1.0, w2[:], op0=ALU.add, op1=ALU.mult
            )
            t = gpool.tile([128, TB], dt.bfloat16, tag="t")
            nc.vector.scalar_tensor_tensor(
                t[:],
                t_i[:],
                1.0,
                r[:],
                op0=ALU.add,
                op1=ALU.mult,
                accum_out=(
                    slab_b[:, bb : bb + 1]
                    if ib == NBB - 1
                    else slab_a[:, bb, ib : ib + 1]
                ),
            )

            if tb == NTB - 2:
                # all slab_a cells are written once block 62 retires — ship
                # them while the final block still streams
                nc.sync.dma_start(out_a[:], slab_a[:])

        nc.sync.dma_start(out_b[:], slab_b[:])

    nc.compile()
    return nc


def _get_nc():
    key = "fp8" if USE_FP8 else "bf16"
    if key not in _CACHE:
        _CACHE[key] = _build_nc(USE_FP8)
    return _CACHE[key]


def _softplus(v):
    return np.log1p(np.exp(-np.abs(v))) + np.maximum(v, 0.0)


def kernel(x, w_in, w_out, b_out, _return_results=False, _trace=False):
    from concourse.bass_utils import run_bass_kernel_spmd

    x = np.asarray(x)
    w_in = np.asarray(w_in)
    w_out = np.asarray(w_out)
    b_out = np.asarray(b_out)

    if USE_FP8:
        cast_dt = ml_dtypes.float8_e4m3  # TRN FP8_EXP4: max ±240, inf above

        def cast(a):
            return np.clip(a, -240.0, 240.0).astype(cast_dt)

        w_scaled = w_in * WSCALE
    else:
        cast_dt = ml_dtypes.bfloat16

        def cast(a):
            return a.astype(cast_dt)

        w_scaled = w_in

    # per-core weight pack: wt[p, kc, g*128+c] = w_scaled[g*H + core*128+c, kc*128+p]
    w5 = w_scaled.reshape(3, N_CORES, CH, KC, 128)
    wts = []
    for c in range(N_CORES):
        wc = np.ascontiguousarray(w5[:, c].transpose(3, 2, 0, 1))  # [128p, KC, 3, CH]
        wts.append(np.asarray(cast(wc)).reshape(128, KC, 384))

    # shared token pack: xt[tb, p, kc, s] = x_flat[tb*TB + s, kc*128 + p]
    xq = cast(x.reshape(TOKS, D))
    xt = np.ascontiguousarray(
        np.asarray(xq).reshape(NTB, TB, KC, 128).transpose(0, 3, 2, 1)
    )

    in_maps = [{"xt": xt, "wt": wts[c]} for c in range(N_CORES)]

    nc = _get_nc()
    # the first execution of a freshly compiled NEFF occasionally hits a
    # transient NRT exec error on this setup — retry once
    try:
        res = run_bass_kernel_spmd(
            nc, in_maps, core_ids=list(range(N_CORES)), trace=_trace
        )
    except Exception:
        import time as _time

        _time.sleep(2.0)
        res = run_bass_kernel_spmd(
            nc, in_maps, core_ids=list(range(N_CORES)), trace=False
        )

    # per core -> channel h = core*128 + p
    Ssum = (
        np.concatenate(
            [
                (
                    np.asarray(r["sums_a"]).astype(np.float64).sum(axis=2)
                    + np.asarray(r["sums_b"]).astype(np.float64)
                ).T
                for r in res.results
            ],
            axis=1,
        )
        * 0.25
    )  # [B, H]

    # exact last-token factor in fp64 (host): log_f[S-1] = -softplus(diff[S-1])
    z_last = x[:, -1, :].astype(np.float64) @ w_in.astype(np.float64).T
    f_l, i_l = z_last[:, :H], z_last[:, H : 2 * H]
    diff_l = _softplus(-f_l) - _softplus(-i_l)
    h_last = np.exp(-_softplus(diff_l) + np.log(0.5 + Ssum))
    out = (h_last @ w_out.astype(np.float64).T + b_out.astype(np.float64)).astype(
        np.float32
    )
    if _return_results:
        return out, res
    return out
